# revision 4
# baseline (speedup 1.0000x reference)
"""Masked-softmax attention on 8 trn2 NeuronCores.

Reference computation (per batch b):
    att = q @ k                        # [n_q, n_k], k given pre-transposed [d, n_k]
    att = where(mask==0, -1e9, att)
    att = softmax(att, -1) / sqrt(d)
    out = (att @ v).T                  # returned [n_dv, n_q]

Sharding: data-parallel over batch: B=16 -> 2 batches per core x 8 cores.

Host-side, per batch, the key dimension is COMPACTED: masked-out keys
contribute exactly 0 to both the softmax numerator and denominator (the
reference's exp(-1e9 - anything) underflows to +0.0 in fp32), so we gather
only the unmasked columns of k / rows of v, padded up to a multiple of 128
(padding killed by the same -1e9 bias). With a Bernoulli(0.5) mask this
halves the contraction length. Exact, not an approximation.

Device-side plan (per batch, all matmuls in float32r = full-rate PE):
    - Work in the TRANSPOSED score layout S^T[k, q] (k on partitions):
        S^T tile [128k, 512q] = k_slice[d,128k]^T @ qT[d, 512q]  (2 d-chunk accum)
      `k` input [d, n_k] is directly the stationary operand; `q` is transposed
      host-side during sharding so qT[d, n_q] is directly the moving operand.
    - softmax is shift-invariant, so instead of the row max we subtract a
      CONSTANT shift (scores ~ N(0, d) with d=256 -> |s| < ~110 always;
      exp(s-shift) can't overflow and dominant terms can't underflow).
      Mask + shift fold into the scalar-engine exp as a per-partition bias:
        e[k, q] = exp(s + bias_k),  bias_k = -shift - 1e9*(1-mask_k)
    - out^T[dv, q] += v_tile[128k, dv_chunk]^T @ e   (v is directly stationary)
      z[dv, q] = sixteens[128k, 128]^T @ acc_e       (= 16Z in EVERY partition:
      the all-16s stationary matrix computes the row sum AND broadcasts it,
      folding in the post-softmax 1/sqrt(d)=1/16 scale). acc_e is a running
      DVE accumulator over ALL the stripe's e-tiles, so ONE Z matmul per
      stripe (PE cycles -> idle DVE).
    - out = out^T * (1/z) (DVE approx reciprocal) -> [dv, n_q], the required
      output layout.

DMA schedule: ALL input DMAs for both batches are issued up-front,
priority-ordered, on the two fast HWDGE queues (SP + Activation issuers);
only batch-1 bulk (k-rest, v) rides the slow-to-post gpsimd SWDGE since it
isn't needed until ~45us in. Head-of-line on the SP queue: k tile 0 + the
first q stripe, so the first real matmul starts ~1.2us after the DMA
engines come up instead of ~7us.
"""

import numpy as np

import concourse.bacc as bacc
import concourse.mybir as mybir
import concourse.tile as tile
from concourse.bass_utils import run_bass_kernel_spmd

P = 128          # partitions
D = 256          # d == n_dv
S = 2048         # n_q
NB = 2           # batches per core
QS = 512         # q-stripe width (max fp32 matmul moving dim)
NQS = S // QS    # 4 q-stripes
NCORES = 8
SHIFT = 60.0     # constant softmax shift (see module docstring)
NWARM = 6        # PE warmup matmuls (fill preamble->first-input window)

F32 = mybir.dt.float32
F32R = mybir.dt.float32r
I32 = mybir.dt.int32
EXP = mybir.ActivationFunctionType.Exp
MULT = mybir.AluOpType.mult
ADD = mybir.AluOpType.add


def build(sk):
    """Build the per-core program. sk = compacted key length (mult of 128)."""
    from contextlib import ExitStack

    nkt = sk // P  # number of k-tiles
    nc = bacc.Bacc()
    qT = nc.declare_dram_parameter("qT", [NB, D, S], F32R, isOutput=False)
    kk = nc.declare_dram_parameter("k", [NB, D, sk], F32R, isOutput=False)
    vv = nc.declare_dram_parameter("v", [NB, sk, D], F32R, isOutput=False)
    bb = nc.declare_dram_parameter("bias", [NB, P, sk // P], F32, isOutput=False)
    out = nc.declare_dram_parameter("out", [NB, D, S], F32, isOutput=True)

    def chunks(lo, hi, n):
        """Split [lo,hi) into n roughly-equal spans (empty spans dropped)."""
        step = max(1, (hi - lo + n - 1) // n)
        return [(a, min(a + step, hi)) for a in range(lo, hi, step)]

    with tile.TileContext(nc) as tc, ExitStack() as ctx:
        consts = ctx.enter_context(tc.tile_pool(name="consts", bufs=1))
        inp = ctx.enter_context(tc.tile_pool(name="inp", bufs=1))
        epool = ctx.enter_context(tc.tile_pool(name="e", bufs=4))
        opool = ctx.enter_context(tc.tile_pool(name="o", bufs=2))
        zpool = ctx.enter_context(tc.tile_pool(name="z", bufs=2))
        ps_s = ctx.enter_context(tc.tile_pool(name="ps_s", bufs=3, space="PSUM"))
        ps_o = ctx.enter_context(tc.tile_pool(name="ps_o", bufs=2, space="PSUM"))
        ps_z = ctx.enter_context(tc.tile_pool(name="ps_z", bufs=1, space="PSUM"))

        # (memset can't emit f32r; stage in f32 and DVE-copy to round)
        sixteens_f = consts.tile([P, P], F32)
        nc.vector.memset(sixteens_f, 16.0)
        sixteens = consts.tile([P, P], F32R)
        nc.vector.tensor_copy(sixteens, sixteens_f)

        # Warmup Exp: walrus attaches the implicit ACT table load to the
        # first Exp, which eats its sync-wait slots; give it a dep-free one
        # (also hides the ~2.7us table load under the input DMA fill).
        warm_in = consts.tile([P, 1], F32)
        nc.vector.memset(warm_in, 0.0)
        warm_out = consts.tile([P, 1], F32)
        nc.scalar.activation(warm_out, warm_in, EXP)

        # ---- all input tiles, both batches resident simultaneously
        kts = [
            [inp.tile([P, sk], F32R, tag=f"k{b}{c}", name=f"kt{b}{c}") for c in range(2)]
            for b in range(NB)
        ]
        qts = [
            [inp.tile([P, S], F32R, tag=f"q{b}{c}", name=f"qt{b}{c}") for c in range(2)]
            for b in range(NB)
        ]
        vts = [inp.tile([P, nkt, D], F32R, tag=f"v{b}", name=f"vt{b}") for b in range(NB)]
        biast = [
            inp.tile([P, nkt], F32, tag=f"bias{b}", name=f"biast{b}") for b in range(NB)
        ]

        # ---- issue ALL input DMAs up-front, priority-ordered.
        # SP HWDGE (fast posting): the whole batch-0 critical path, then
        # batch-1's k tile 0 + q. Head-of-line = k tile 0 + q stripe 0.
        def k_tile_dma(eng, b, c, t0, t1):
            eng.dma_start(
                out=kts[b][c][:, t0 * P : t1 * P], in_=kk[b, c * P : (c + 1) * P, t0 * P : t1 * P]
            )

        def q_stripe_dma(eng, b, s, c):
            eng.dma_start(
                out=qts[b][c][:, s * QS : (s + 1) * QS],
                in_=qT[b, c * P : (c + 1) * P, s * QS : (s + 1) * QS],
            )

        def v_chunk_dma(eng, b, t0, t1):
            eng.dma_start(
                out=vts[b][:, t0:t1, :],
                in_=vv[b, t0 * P : t1 * P, :].rearrange("(t p) d -> p t d", p=P),
            )

        v0_chunks = chunks(0, nkt, 3)
        for c in range(2):
            k_tile_dma(nc.sync, 0, c, 0, 1)
        for c in range(2):
            q_stripe_dma(nc.sync, 0, 0, c)
        v_chunk_dma(nc.sync, 0, *v0_chunks[0])
        for s in range(1, NQS):
            for c in range(2):
                q_stripe_dma(nc.sync, 0, s, c)
            if s < len(v0_chunks):
                v_chunk_dma(nc.sync, 0, *v0_chunks[s])
        for c in range(2):
            k_tile_dma(nc.sync, 1, c, 0, 1)
        for s in range(NQS):
            for c in range(2):
                q_stripe_dma(nc.sync, 1, s, c)

        # ACT HWDGE: biases + batch-0 k rest (needed within ~2-6us).
        nc.scalar.dma_start(out=biast[0], in_=bb[0])
        if nkt > 1:
            for t0, t1 in chunks(1, nkt, 2):
                for c in range(2):
                    k_tile_dma(nc.scalar, 0, c, t0, t1)
        nc.scalar.dma_start(out=biast[1], in_=bb[1])

        # gpsimd SWDGE (slow ~2.4us/post but off the critical path):
        # batch-1 bulk, not consumed until ~45us.
        if nkt > 1:
            for c in range(2):
                k_tile_dma(nc.gpsimd, 1, c, 1, nkt)
        for t0, t1 in chunks(0, nkt, 2):
            v_chunk_dma(nc.gpsimd, 1, t0, t1)

        # PE warmup: dep-free matmuls during the initial DMA fill so the HAM
        # clock gate ramps before the real matmuls start.
        for w in range(NWARM):
            wp = ps_s.tile([P, P], F32, tag="s", name=f"warm{w}")
            nc.tensor.matmul(wp, lhsT=sixteens, rhs=sixteens, start=True, stop=True)

        # ---- compute, one 512-wide q-stripe at a time
        for b in range(NB):
            for s in range(NQS):
                last_stripe = b == NB - 1 and s == NQS - 1
                qoff, qw = s * QS, QS
                qsl = slice(qoff, qoff + qw)
                op0 = ps_o.tile([P, QS], F32, tag="o0", name="op0")[:, :qw]
                op1 = ps_o.tile([P, QS], F32, tag="o1", name="op1")[:, :qw]
                zp = ps_z.tile([P, QS], F32, tag="z", name="zp")[:, :qw]
                acc_e = None
                e_last = None
                for t in range(nkt):
                    ksl = slice(t * P, (t + 1) * P)
                    sp = ps_s.tile([P, QS], F32, tag="s", name="sp")[:, :qw]
                    nc.tensor.matmul(
                        sp, lhsT=kts[b][0][:, ksl], rhs=qts[b][0][:, qsl],
                        start=True, stop=False,
                    )
                    nc.tensor.matmul(
                        sp, lhsT=kts[b][1][:, ksl], rhs=qts[b][1][:, qsl],
                        start=False, stop=True,
                    )
                    e = epool.tile([P, QS], F32R, tag="e", name="e")[:, :qw]
                    nc.scalar.activation(e, sp, EXP, bias=biast[b][:, t : t + 1])
                    first, last = t == 0, t == nkt - 1
                    nc.tensor.matmul(
                        op0, lhsT=vts[b][:, t, 0:P], rhs=e, start=first, stop=last,
                    )
                    nc.tensor.matmul(
                        op1, lhsT=vts[b][:, t, P : 2 * P], rhs=e, start=first, stop=last,
                    )
                    # Z: running DVE accumulator over the stripe's e-tiles ->
                    # a single Z matmul per stripe. On the final stripe the
                    # last e-tile goes into a second accumulating Z matmul
                    # instead, cutting one DVE add off the drain chain.
                    if last_stripe and last:
                        nc.tensor.matmul(
                            zp, lhsT=sixteens, rhs=acc_e, start=True, stop=False,
                        )
                        nc.tensor.matmul(
                            zp, lhsT=sixteens, rhs=e, start=False, stop=True,
                        )
                    elif acc_e is None:
                        acc_e = e
                    else:
                        na = epool.tile([P, QS], F32R, tag="ep", name="na")[:, :qw]
                        nc.vector.tensor_tensor(na, acc_e, e, ADD)
                        acc_e = na
                if not last_stripe:
                    nc.tensor.matmul(zp, lhsT=sixteens, rhs=acc_e, start=True, stop=True)
                # normalize: out = out_unnorm * (1/(16Z)); zp already holds
                # 16Z in every partition. ~18-bit reciprocal, 5x faster than
                # exact; z is far from denorm/inf so approx edge cases can't
                # hit. Processed in chunks so the tail (recip -> mult -> DMA)
                # pipelines; the final stripe uses finer chunks and puts one
                # multiply stream on the Pool engine to shorten the drain.
                zbs = zpool.tile([P, QS], F32, tag="zbs", name="zbs")[:, :qw]
                o0 = opool.tile([P, QS], F32, tag="so0", name="o0")[:, :qw]
                o1 = opool.tile([P, QS], F32, tag="so1", name="o1")[:, :qw]
                nch = 4 if last_stripe else 2
                cw = qw // nch
                for h in range(nch):
                    hs = slice(h * cw, (h + 1) * cw)
                    oqsl = slice(qoff + h * cw, qoff + (h + 1) * cw)
                    nc.vector.reciprocal_approx_fast(out=zbs[:, hs], in_=zp[:, hs])
                    nc.vector.tensor_tensor(o0[:, hs], op0[:, hs], zbs[:, hs], MULT)
                    nc.vector.tensor_tensor(o1[:, hs], op1[:, hs], zbs[:, hs], MULT)
                    nc.sync.dma_start(out=out[b, 0:P, oqsl], in_=o0[:, hs])
                    nc.scalar.dma_start(out=out[b, P : 2 * P, oqsl], in_=o1[:, hs])

    return nc


def make_in_maps(q, k, v, mask):
    """Shard over batch; transpose q; compact the key dim to unmasked keys."""
    q = np.asarray(q, dtype=np.float32)
    k = np.asarray(k, dtype=np.float32)
    v = np.asarray(v, dtype=np.float32)
    mask = np.asarray(mask, dtype=np.int32).reshape(len(q), -1)

    B = len(q)
    idxs = [np.nonzero(mask[b])[0] for b in range(B)]
    n_eff = max((len(ix) for ix in idxs), default=1)
    sk = max(P, ((n_eff + P - 1) // P) * P)  # padded compacted key length

    kg = np.zeros((B, D, sk), dtype=np.float32)
    vg = np.zeros((B, sk, D), dtype=np.float32)
    # exp bias: -SHIFT for real keys, -1e9 for padding (kills it exactly),
    # laid out [P, sk//P] partition-major to match the k-tile slicing
    bg = np.full((B, sk), -1.0e9, dtype=np.float32)
    for b in range(B):
        ix = idxs[b]
        kg[b, :, : len(ix)] = k[b][:, ix]
        vg[b, : len(ix)] = v[b][ix]
        bg[b, : len(ix)] = -SHIFT
    bgt = np.ascontiguousarray(
        bg.reshape(B, sk // P, P).transpose(0, 2, 1)
    )  # [B, P, nkt]

    in_maps = []
    for i in range(NCORES):
        sl = slice(i * NB, (i + 1) * NB)
        in_maps.append(
            {
                "qT": np.ascontiguousarray(np.transpose(q[sl], (0, 2, 1))),
                "k": np.ascontiguousarray(kg[sl]),
                "v": np.ascontiguousarray(vg[sl]),
                "bias": np.ascontiguousarray(bgt[sl]),
            }
        )
    return in_maps, sk


def run(q, k, v, mask, **kwargs):
    in_maps, sk = make_in_maps(q, k, v, mask)
    nc = build(sk)
    nc.finalize()  # run the Bacc pass pipeline (reg alloc, wait splitting)
    res = run_bass_kernel_spmd(nc, in_maps, list(range(NCORES)), **kwargs)
    out = np.concatenate([r["out"] for r in res.results], axis=0)
    return out, res


def kernel(q, k, v, mask):
    out, _ = run(q, k, v, mask)
    return out


# revision 5
# speedup vs baseline: 1.2256x; 1.2256x over previous
"""Masked-softmax attention on 8 trn2 NeuronCores.

Reference computation (per batch b):
    att = q @ k                        # [n_q, n_k], k given pre-transposed [d, n_k]
    att = where(mask==0, -1e9, att)
    att = softmax(att, -1) / sqrt(d)
    out = (att @ v).T                  # returned [n_dv, n_q]

Sharding: data-parallel over batch: B=16 -> 2 batches per core x 8 cores.

Host-side, per batch, the key dimension is COMPACTED: masked-out keys
contribute exactly 0 to both the softmax numerator and denominator (the
reference's exp(-1e9 - anything) underflows to +0.0 in fp32), so we gather
only the unmasked columns of k / rows of v, padded up to a multiple of 128
(padding killed by the same -1e9 bias). With a Bernoulli(0.5) mask this
halves the contraction length. Exact, not an approximation.

Device-side plan (per batch). All matmul OPERANDS are bf16 (accumulation
stays fp32 in PSUM): on TRN2's PE both bf16 and f32r run 1 cycle/row at
512-wide moving, but bf16 halves every SBUF fetch and all input DMA, which
removes the SBUF-port contention between the PE's moving-operand stream
and the DVE's e-accumulation traffic (measured: f32r matmul spacing
degrades 232->278 ns when DVE traffic rises; bf16 keeps the PE at rate).
bf16 rounding (~0.2% rms on scores -> ~2.6% per softmax weight averaged
over ~1e3 keys) lands ~1e-3 relative on the output, far inside the 2e-2
gate.

    - Work in the TRANSPOSED score layout S^T[k, q] (k on partitions):
        S^T tile [128k, 512q] = k_slice[d,128k]^T @ qT[d, 512q]  (2 d-chunk accum)
      `k` input [d, n_k] is directly the stationary operand; `q` is transposed
      host-side during sharding so qT[d, n_q] is directly the moving operand.
    - softmax is shift-invariant, so instead of the row max we subtract a
      CONSTANT shift (scores ~ N(0, d) with d=256 -> |s| < ~110 always;
      exp(s-shift) can't overflow and dominant terms can't underflow).
      Mask + shift fold into the scalar-engine exp as a per-partition bias:
        e[k, q] = exp(s + bias_k),  bias_k = -shift - 1e9*(1-mask_k)
    - out^T[dv, q] += v_tile[128k, dv_chunk]^T @ e   (v is directly stationary)
      z[dv, q]    += sixteens[128k, 128]^T @ e       (= 16Z in EVERY partition:
      the all-16s stationary matrix computes the row sum AND broadcasts it,
      folding in the post-softmax 1/sqrt(d)=1/16 scale)
    - out = out^T * (1/z) (DVE approx reciprocal) -> [dv, n_q], the required
      output layout.

DMA schedule: ALL input DMAs for both batches are issued up-front on large,
full-line transfers. SP HWDGE: biases + q stripes (stripe 0 first).
ACT HWDGE: batch-0 k halves (whole [128, sk] rows, best DMA efficiency).
gpsimd SWDGE (slow to post but off the critical path): v chunks + batch-1
k. Outputs ride SP/ACT, issued during compute after all input posts.
"""

import numpy as np
import ml_dtypes

import concourse.bacc as bacc
import concourse.mybir as mybir
import concourse.tile as tile
from concourse.bass_utils import run_bass_kernel_spmd

P = 128          # partitions
D = 256          # d == n_dv
S = 2048         # n_q
NB = 2           # batches per core
QS = 512         # q-stripe width (max matmul moving dim into one PSUM bank)
NQS = S // QS    # 4 q-stripes
NCORES = 8
SHIFT = 60.0     # constant softmax shift (see module docstring)
NWARM = 8        # PE warmup matmuls (fill preamble->first-input window)
QUAD = 4         # e-tiles accumulated on DVE per Z matmul

F32 = mybir.dt.float32
BF16 = mybir.dt.bfloat16
EXP = mybir.ActivationFunctionType.Exp
MULT = mybir.AluOpType.mult
ADD = mybir.AluOpType.add

BF16NP = ml_dtypes.bfloat16


def build(sk):
    """Build the per-core program. sk = compacted key length (mult of 128)."""
    from contextlib import ExitStack

    nkt = sk // P  # number of k-tiles
    nc = bacc.Bacc()
    qT = nc.declare_dram_parameter("qT", [NB, D, S], BF16, isOutput=False)
    kk = nc.declare_dram_parameter("k", [NB, D, sk], BF16, isOutput=False)
    vv = nc.declare_dram_parameter("v", [NB, sk, D], BF16, isOutput=False)
    bb = nc.declare_dram_parameter("bias", [NB, P, sk // P], F32, isOutput=False)
    out = nc.declare_dram_parameter("out", [NB, D, S], F32, isOutput=True)

    def chunks(lo, hi, n):
        """Split [lo,hi) into n roughly-equal spans (empty spans dropped)."""
        step = max(1, (hi - lo + n - 1) // n)
        return [(a, min(a + step, hi)) for a in range(lo, hi, step)]

    with tile.TileContext(nc) as tc, ExitStack() as ctx:
        consts = ctx.enter_context(tc.tile_pool(name="consts", bufs=1))
        inp = ctx.enter_context(tc.tile_pool(name="inp", bufs=1))
        epool = ctx.enter_context(tc.tile_pool(name="e", bufs=4))
        opool = ctx.enter_context(tc.tile_pool(name="o", bufs=2))
        zpool = ctx.enter_context(tc.tile_pool(name="z", bufs=2))
        ps_s = ctx.enter_context(tc.tile_pool(name="ps_s", bufs=3, space="PSUM"))
        ps_o = ctx.enter_context(tc.tile_pool(name="ps_o", bufs=2, space="PSUM"))
        ps_z = ctx.enter_context(tc.tile_pool(name="ps_z", bufs=1, space="PSUM"))

        # (memset can't emit bf16; stage in f32 and DVE-copy to round)
        sixteens_f = consts.tile([P, P], F32)
        nc.vector.memset(sixteens_f, 16.0)
        sixteens = consts.tile([P, P], BF16)
        nc.vector.tensor_copy(sixteens, sixteens_f)

        # Warmup Exp: walrus attaches the implicit ACT table load to the
        # first Exp, which eats its sync-wait slots; give it a dep-free one
        # (also hides the ~2.7us table load under the input DMA fill).
        warm_in = consts.tile([P, 1], F32)
        nc.vector.memset(warm_in, 0.0)
        warm_out = consts.tile([P, 1], F32)
        nc.scalar.activation(warm_out, warm_in, EXP)

        # ---- all input tiles, both batches resident simultaneously
        kts = [
            [inp.tile([P, sk], BF16, tag=f"k{b}{c}", name=f"kt{b}{c}") for c in range(2)]
            for b in range(NB)
        ]
        qts = [
            [inp.tile([P, S], BF16, tag=f"q{b}{c}", name=f"qt{b}{c}") for c in range(2)]
            for b in range(NB)
        ]
        vts = [inp.tile([P, nkt, D], BF16, tag=f"v{b}", name=f"vt{b}") for b in range(NB)]
        biast = [
            inp.tile([P, nkt], F32, tag=f"bias{b}", name=f"biast{b}") for b in range(NB)
        ]

        # ---- issue ALL input DMAs up-front, priority-ordered, large
        # full-line transfers only. Posting a descriptor costs the issuing
        # engine ~0.7-0.9us, so critical queues carry few posts.
        def q_stripe_dma(eng, b, s, c):
            eng.dma_start(
                out=qts[b][c][:, s * QS : (s + 1) * QS],
                in_=qT[b, c * P : (c + 1) * P, s * QS : (s + 1) * QS],
            )

        def v_chunk_dma(eng, b, t0, t1):
            eng.dma_start(
                out=vts[b][:, t0:t1, :],
                in_=vv[b, t0 * P : t1 * P, :].rearrange("(t p) d -> p t d", p=P),
            )

        # SP HWDGE: biases (tiny) then q stripes, batch 0 stripe 0 first.
        nc.sync.dma_start(out=biast[0], in_=bb[0])
        nc.sync.dma_start(out=biast[1], in_=bb[1])
        for b in range(NB):
            for s in range(NQS):
                for c in range(2):
                    q_stripe_dma(nc.sync, b, s, c)

        # ACT HWDGE: batch-0 k halves as whole-row transfers.
        for c in range(2):
            nc.scalar.dma_start(out=kts[0][c], in_=kk[0, c * P : (c + 1) * P, :])

        # gpsimd SWDGE: v + batch-1 k (not consumed until ~45us).
        for t0, t1 in chunks(0, nkt, 3):
            v_chunk_dma(nc.gpsimd, 0, t0, t1)
        for c in range(2):
            nc.gpsimd.dma_start(out=kts[1][c], in_=kk[1, c * P : (c + 1) * P, :])
        for t0, t1 in chunks(0, nkt, 2):
            v_chunk_dma(nc.gpsimd, 1, t0, t1)

        # PE warmup: dep-free matmuls during the initial DMA fill so the HAM
        # clock gate ramps before the real matmuls start.
        for w in range(NWARM):
            wp = ps_s.tile([P, P], F32, tag="s", name=f"warm{w}")
            nc.tensor.matmul(wp, lhsT=sixteens, rhs=sixteens, start=True, stop=True)

        # ---- compute, one 512-wide q-stripe at a time
        for b in range(NB):
            for s in range(NQS):
                last_stripe = b == NB - 1 and s == NQS - 1
                qoff, qw = s * QS, QS
                qsl = slice(qoff, qoff + qw)
                op0 = ps_o.tile([P, QS], F32, tag="o0", name="op0")[:, :qw]
                op1 = ps_o.tile([P, QS], F32, tag="o1", name="op1")[:, :qw]
                zp = ps_z.tile([P, QS], F32, tag="z", name="zp")[:, :qw]
                acc_e = None
                nacc = 0
                nzmm = (nkt + QUAD - 1) // QUAD
                zi = 0
                for t in range(nkt):
                    ksl = slice(t * P, (t + 1) * P)
                    sp = ps_s.tile([P, QS], F32, tag="s", name="sp")[:, :qw]
                    nc.tensor.matmul(
                        sp, lhsT=kts[b][0][:, ksl], rhs=qts[b][0][:, qsl],
                        start=True, stop=False,
                    )
                    nc.tensor.matmul(
                        sp, lhsT=kts[b][1][:, ksl], rhs=qts[b][1][:, qsl],
                        start=False, stop=True,
                    )
                    e = epool.tile([P, QS], BF16, tag="e", name="e")[:, :qw]
                    nc.scalar.activation(e, sp, EXP, bias=biast[b][:, t : t + 1])
                    first, last = t == 0, t == nkt - 1
                    nc.tensor.matmul(
                        op0, lhsT=vts[b][:, t, 0:P], rhs=e, start=first, stop=last,
                    )
                    nc.tensor.matmul(
                        op1, lhsT=vts[b][:, t, P : 2 * P], rhs=e, start=first, stop=last,
                    )
                    # Z: a running DVE accumulator sums QUAD e-tiles so only
                    # ceil(nkt/QUAD) Z matmuls run (PE cycles -> idle DVE)
                    if acc_e is None:
                        acc_e, nacc = e, 1
                    else:
                        na = epool.tile([P, QS], BF16, tag="ep", name="na")[:, :qw]
                        nc.vector.tensor_tensor(na, acc_e, e, ADD)
                        acc_e = na
                        nacc += 1
                    if nacc == QUAD or t == nkt - 1:
                        nc.tensor.matmul(
                            zp, lhsT=sixteens, rhs=acc_e,
                            start=zi == 0, stop=zi == nzmm - 1,
                        )
                        zi += 1
                        acc_e, nacc = None, 0
                # normalize: out = out_unnorm * (1/(16Z)); zp already holds
                # 16Z in every partition. ~18-bit reciprocal, 5x faster than
                # exact; z is far from denorm/inf so approx edge cases can't
                # hit. Processed in chunks so the tail (recip -> mult -> DMA)
                # pipelines; the final stripe uses finer chunks to shorten
                # the drain.
                zbs = zpool.tile([P, QS], F32, tag="zbs", name="zbs")[:, :qw]
                o0 = opool.tile([P, QS], F32, tag="so0", name="o0")[:, :qw]
                o1 = opool.tile([P, QS], F32, tag="so1", name="o1")[:, :qw]
                nch = 4 if last_stripe else 2
                cw = qw // nch
                for h in range(nch):
                    hs = slice(h * cw, (h + 1) * cw)
                    oqsl = slice(qoff + h * cw, qoff + (h + 1) * cw)
                    nc.vector.reciprocal_approx_fast(out=zbs[:, hs], in_=zp[:, hs])
                    nc.vector.tensor_tensor(o0[:, hs], op0[:, hs], zbs[:, hs], MULT)
                    nc.vector.tensor_tensor(o1[:, hs], op1[:, hs], zbs[:, hs], MULT)
                    nc.sync.dma_start(out=out[b, 0:P, oqsl], in_=o0[:, hs])
                    nc.scalar.dma_start(out=out[b, P : 2 * P, oqsl], in_=o1[:, hs])

    return nc


def make_in_maps(q, k, v, mask):
    """Shard over batch; transpose q; compact the key dim to unmasked keys."""
    q = np.asarray(q, dtype=np.float32)
    k = np.asarray(k, dtype=np.float32)
    v = np.asarray(v, dtype=np.float32)
    mask = np.asarray(mask, dtype=np.int32).reshape(len(q), -1)

    B = len(q)
    idxs = [np.nonzero(mask[b])[0] for b in range(B)]
    n_eff = max((len(ix) for ix in idxs), default=1)
    sk = max(P, ((n_eff + P - 1) // P) * P)  # padded compacted key length

    kg = np.zeros((B, D, sk), dtype=np.float32)
    vg = np.zeros((B, sk, D), dtype=np.float32)
    # exp bias: -SHIFT for real keys, -1e9 for padding (kills it exactly),
    # laid out [P, sk//P] partition-major to match the k-tile slicing
    bg = np.full((B, sk), -1.0e9, dtype=np.float32)
    for b in range(B):
        ix = idxs[b]
        kg[b, :, : len(ix)] = k[b][:, ix]
        vg[b, : len(ix)] = v[b][ix]
        bg[b, : len(ix)] = -SHIFT
    bgt = np.ascontiguousarray(
        bg.reshape(B, sk // P, P).transpose(0, 2, 1)
    )  # [B, P, nkt]

    in_maps = []
    for i in range(NCORES):
        sl = slice(i * NB, (i + 1) * NB)
        in_maps.append(
            {
                "qT": np.ascontiguousarray(
                    np.transpose(q[sl], (0, 2, 1)).astype(BF16NP)
                ),
                "k": np.ascontiguousarray(kg[sl].astype(BF16NP)),
                "v": np.ascontiguousarray(vg[sl].astype(BF16NP)),
                "bias": np.ascontiguousarray(bgt[sl]),
            }
        )
    return in_maps, sk


def run(q, k, v, mask, **kwargs):
    in_maps, sk = make_in_maps(q, k, v, mask)
    nc = build(sk)
    nc.finalize()  # run the Bacc pass pipeline (reg alloc, wait splitting)
    res = run_bass_kernel_spmd(nc, in_maps, list(range(NCORES)), **kwargs)
    out = np.concatenate([r["out"] for r in res.results], axis=0)
    return out, res


def kernel(q, k, v, mask):
    out, _ = run(q, k, v, mask)
    return out


# revision 7
# speedup vs baseline: 1.2969x; 1.0582x over previous
"""Masked-softmax attention on 8 trn2 NeuronCores.

Reference computation (per batch b):
    att = q @ k                        # [n_q, n_k], k given pre-transposed [d, n_k]
    att = where(mask==0, -1e9, att)
    att = softmax(att, -1) / sqrt(d)
    out = (att @ v).T                  # returned [n_dv, n_q]

Sharding: data-parallel over batch: B=16 -> 2 batches per core x 8 cores.

Host-side, per batch, the key dimension is COMPACTED: masked-out keys
contribute exactly 0 to both the softmax numerator and denominator (the
reference's exp(-1e9 - anything) underflows to +0.0 in fp32), so we gather
only the unmasked columns of k / rows of v, padded up to a multiple of 128
(padding killed by the same -1e9 bias). With a Bernoulli(0.5) mask this
halves the contraction length. Exact, not an approximation.

Device-side plan (per batch). All matmul OPERANDS are bf16 (accumulation
stays fp32 in PSUM): on TRN2's PE both bf16 and f32r run 1 cycle/row at
512-wide moving, but bf16 halves every SBUF fetch and all input DMA, which
removes the SBUF-port contention between the PE's moving-operand stream
and the DVE's e-accumulation traffic (measured: f32r matmul spacing
degrades 232->278 ns when DVE traffic rises; bf16 keeps the PE at rate).
bf16 rounding (~0.2% rms on scores -> ~2.6% per softmax weight averaged
over ~1e3 keys) lands ~1e-3 relative on the output, far inside the 2e-2
gate.

    - Work in the TRANSPOSED score layout S^T[k, q] (k on partitions):
        S^T tile [128k, 512q] = k_slice[d,128k]^T @ qT[d, 512q]  (2 d-chunk accum)
      `k` input [d, n_k] is directly the stationary operand; `q` is transposed
      host-side during sharding so qT[d, n_q] is directly the moving operand.
    - softmax is shift-invariant, so instead of the row max we subtract a
      CONSTANT shift (scores ~ N(0, d) with d=256 -> |s| < ~110 always;
      exp(s-shift) can't overflow and dominant terms can't underflow).
      Mask + shift fold into the scalar-engine exp as a per-partition bias:
        e[k, q] = exp(s + bias_k),  bias_k = -shift - 1e9*(1-mask_k)
    - out^T[dv, q] += v_tile[128k, dv_chunk]^T @ e   (v is directly stationary)
      z[dv, q]    += sixteens[128k, 128]^T @ e       (= 16Z in EVERY partition:
      the all-16s stationary matrix computes the row sum AND broadcasts it,
      folding in the post-softmax 1/sqrt(d)=1/16 scale)
    - out = out^T * (1/z) (DVE approx reciprocal) -> [dv, n_q], the required
      output layout.

DMA schedule: ALL input DMAs for both batches are issued up-front on large,
full-line transfers. SP HWDGE: biases + q stripes (stripe 0 first).
ACT HWDGE: batch-0 k halves (whole [128, sk] rows, best DMA efficiency).
gpsimd SWDGE (slow to post but off the critical path): v chunks + batch-1
k. Outputs ride SP/ACT, issued during compute after all input posts.
"""

import numpy as np
import ml_dtypes

import concourse.bacc as bacc
import concourse.mybir as mybir
import concourse.tile as tile
from concourse.bass_utils import run_bass_kernel_spmd

P = 128          # partitions
D = 256          # d == n_dv
S = 2048         # n_q
NB = 2           # batches per core
QS = 512         # q-stripe width (max matmul moving dim into one PSUM bank)
NQS = S // QS    # 4 q-stripes
NCORES = 8
SHIFT = 60.0     # constant softmax shift (see module docstring)
NWARM = 16       # PE warmup matmuls (fill preamble->first-input window)
QUAD = 16        # e-tiles accumulated on DVE per Z matmul (>=nkt: 1 Z mm/stripe)

F32 = mybir.dt.float32
BF16 = mybir.dt.bfloat16
EXP = mybir.ActivationFunctionType.Exp
MULT = mybir.AluOpType.mult
ADD = mybir.AluOpType.add

BF16NP = ml_dtypes.bfloat16


def build(sk):
    """Build the per-core program. sk = compacted key length (mult of 128)."""
    from contextlib import ExitStack

    nkt = sk // P  # number of k-tiles
    nc = bacc.Bacc()
    qT = nc.declare_dram_parameter("qT", [NB, D, S], BF16, isOutput=False)
    kk = nc.declare_dram_parameter("k", [NB, D, sk], BF16, isOutput=False)
    vv = nc.declare_dram_parameter("v", [NB, sk, D], BF16, isOutput=False)
    bb = nc.declare_dram_parameter("bias", [NB, P, sk // P], F32, isOutput=False)
    out = nc.declare_dram_parameter("out", [NB, D, S], F32, isOutput=True)

    def chunks(lo, hi, n):
        """Split [lo,hi) into n roughly-equal spans (empty spans dropped)."""
        step = max(1, (hi - lo + n - 1) // n)
        return [(a, min(a + step, hi)) for a in range(lo, hi, step)]

    with tile.TileContext(nc) as tc, ExitStack() as ctx:
        consts = ctx.enter_context(tc.tile_pool(name="consts", bufs=1))
        inp = ctx.enter_context(tc.tile_pool(name="inp", bufs=1))
        epool = ctx.enter_context(tc.tile_pool(name="e", bufs=4))
        opool = ctx.enter_context(tc.tile_pool(name="o", bufs=2))
        zpool = ctx.enter_context(tc.tile_pool(name="z", bufs=2))
        ps_s = ctx.enter_context(tc.tile_pool(name="ps_s", bufs=3, space="PSUM"))
        ps_o = ctx.enter_context(tc.tile_pool(name="ps_o", bufs=2, space="PSUM"))
        ps_z = ctx.enter_context(tc.tile_pool(name="ps_z", bufs=1, space="PSUM"))

        # (memset can't emit bf16; stage in f32 and DVE-copy to round)
        sixteens_f = consts.tile([P, P], F32)
        nc.vector.memset(sixteens_f, 16.0)
        sixteens = consts.tile([P, P], BF16)
        nc.vector.tensor_copy(sixteens, sixteens_f)

        # Warmup Exp: walrus attaches the implicit ACT table load to the
        # first Exp, which eats its sync-wait slots; give it a dep-free one
        # (also hides the ~2.7us table load under the input DMA fill).
        warm_in = consts.tile([P, 1], F32)
        nc.vector.memset(warm_in, 0.0)
        warm_out = consts.tile([P, 1], F32)
        nc.scalar.activation(warm_out, warm_in, EXP)

        # ---- all input tiles, both batches resident simultaneously
        kts = [
            [inp.tile([P, sk], BF16, tag=f"k{b}{c}", name=f"kt{b}{c}") for c in range(2)]
            for b in range(NB)
        ]
        qts = [
            [inp.tile([P, S], BF16, tag=f"q{b}{c}", name=f"qt{b}{c}") for c in range(2)]
            for b in range(NB)
        ]
        vts = [inp.tile([P, nkt, D], BF16, tag=f"v{b}", name=f"vt{b}") for b in range(NB)]
        biast = [
            inp.tile([P, nkt], F32, tag=f"bias{b}", name=f"biast{b}") for b in range(NB)
        ]

        # ---- issue ALL input DMAs up-front, priority-ordered, large
        # full-line transfers only. Posting a descriptor costs the issuing
        # engine ~0.7-0.9us, so critical queues carry few posts.
        def q_stripe_dma(eng, b, s, c):
            eng.dma_start(
                out=qts[b][c][:, s * QS : (s + 1) * QS],
                in_=qT[b, c * P : (c + 1) * P, s * QS : (s + 1) * QS],
            )

        def v_chunk_dma(eng, b, t0, t1):
            eng.dma_start(
                out=vts[b][:, t0:t1, :],
                in_=vv[b, t0 * P : t1 * P, :].rearrange("(t p) d -> p t d", p=P),
            )

        # SP HWDGE: q stripes (batch 0 stripe 0 first); biases (tiny rows,
        # slow trickle transfers) ride BEHIND stripe 0, never head-of-line.
        for c in range(2):
            q_stripe_dma(nc.sync, 0, 0, c)
        nc.sync.dma_start(out=biast[0], in_=bb[0])
        for c in range(2):
            q_stripe_dma(nc.sync, 0, 1, c)
        nc.sync.dma_start(out=biast[1], in_=bb[1])
        for s in range(2, NQS):
            for c in range(2):
                q_stripe_dma(nc.sync, 0, s, c)
        for s in range(NQS):
            for c in range(2):
                q_stripe_dma(nc.sync, 1, s, c)

        # ACT HWDGE: batch-0 k, both d-halves' leading tiles first so the
        # first S accumulation pair completes ASAP.
        for t0, t1 in chunks(0, nkt, 2):
            for c in range(2):
                nc.scalar.dma_start(
                    out=kts[0][c][:, t0 * P : t1 * P],
                    in_=kk[0, c * P : (c + 1) * P, t0 * P : t1 * P],
                )

        # gpsimd SWDGE: v first (O needs v tile 0 by ~11us), then batch-1
        # bulk (not consumed until ~45us).
        for t0, t1 in chunks(0, nkt, 3):
            v_chunk_dma(nc.gpsimd, 0, t0, t1)
        for c in range(2):
            nc.gpsimd.dma_start(out=kts[1][c], in_=kk[1, c * P : (c + 1) * P, :])
        for t0, t1 in chunks(0, nkt, 2):
            v_chunk_dma(nc.gpsimd, 1, t0, t1)

        # PE warmup: dep-free matmuls during the initial DMA fill so the HAM
        # clock gate ramps before the real matmuls start.
        for w in range(NWARM):
            wp = ps_s.tile([P, P], F32, tag="s", name=f"warm{w}")
            nc.tensor.matmul(wp, lhsT=sixteens, rhs=sixteens, start=True, stop=True)

        # ---- compute, one 512-wide q-stripe at a time
        for b in range(NB):
            for s in range(NQS):
                last_stripe = b == NB - 1 and s == NQS - 1
                qoff, qw = s * QS, QS
                qsl = slice(qoff, qoff + qw)
                op0 = ps_o.tile([P, QS], F32, tag="o0", name="op0")[:, :qw]
                op1 = ps_o.tile([P, QS], F32, tag="o1", name="op1")[:, :qw]
                zp = ps_z.tile([P, QS], F32, tag="z", name="zp")[:, :qw]
                acc_e = None
                nacc = 0
                nzmm = (nkt + QUAD - 1) // QUAD
                zi = 0
                for t in range(nkt):
                    ksl = slice(t * P, (t + 1) * P)
                    sp = ps_s.tile([P, QS], F32, tag="s", name="sp")[:, :qw]
                    nc.tensor.matmul(
                        sp, lhsT=kts[b][0][:, ksl], rhs=qts[b][0][:, qsl],
                        start=True, stop=False,
                    )
                    nc.tensor.matmul(
                        sp, lhsT=kts[b][1][:, ksl], rhs=qts[b][1][:, qsl],
                        start=False, stop=True,
                    )
                    e = epool.tile([P, QS], BF16, tag="e", name="e")[:, :qw]
                    nc.scalar.activation(e, sp, EXP, bias=biast[b][:, t : t + 1])
                    first, last = t == 0, t == nkt - 1
                    nc.tensor.matmul(
                        op0, lhsT=vts[b][:, t, 0:P], rhs=e, start=first, stop=last,
                    )
                    nc.tensor.matmul(
                        op1, lhsT=vts[b][:, t, P : 2 * P], rhs=e, start=first, stop=last,
                    )
                    # Z: a running DVE accumulator sums QUAD e-tiles so only
                    # ceil(nkt/QUAD) Z matmuls run (PE cycles -> idle DVE)
                    if acc_e is None:
                        acc_e, nacc = e, 1
                    else:
                        na = epool.tile([P, QS], BF16, tag="ep", name="na")[:, :qw]
                        nc.vector.tensor_tensor(na, acc_e, e, ADD)
                        acc_e = na
                        nacc += 1
                    if nacc == QUAD or t == nkt - 1:
                        nc.tensor.matmul(
                            zp, lhsT=sixteens, rhs=acc_e,
                            start=zi == 0, stop=zi == nzmm - 1,
                        )
                        zi += 1
                        acc_e, nacc = None, 0
                # normalize: out = out_unnorm * (1/(16Z)); zp already holds
                # 16Z in every partition. ~18-bit reciprocal, 5x faster than
                # exact; z is far from denorm/inf so approx edge cases can't
                # hit. Processed in chunks so the tail (recip -> mult -> DMA)
                # pipelines; the final stripe uses finer chunks to shorten
                # the drain.
                zbs = zpool.tile([P, QS], F32, tag="zbs", name="zbs")[:, :qw]
                o0 = opool.tile([P, QS], F32, tag="so0", name="o0")[:, :qw]
                o1 = opool.tile([P, QS], F32, tag="so1", name="o1")[:, :qw]
                nch = 4 if last_stripe else 2
                cw = qw // nch
                for h in range(nch):
                    hs = slice(h * cw, (h + 1) * cw)
                    oqsl = slice(qoff + h * cw, qoff + (h + 1) * cw)
                    nc.vector.reciprocal_approx_fast(out=zbs[:, hs], in_=zp[:, hs])
                    nc.vector.tensor_tensor(o0[:, hs], op0[:, hs], zbs[:, hs], MULT)
                    nc.vector.tensor_tensor(o1[:, hs], op1[:, hs], zbs[:, hs], MULT)
                    nc.sync.dma_start(out=out[b, 0:P, oqsl], in_=o0[:, hs])
                    nc.scalar.dma_start(out=out[b, P : 2 * P, oqsl], in_=o1[:, hs])

    return nc


def make_in_maps(q, k, v, mask):
    """Shard over batch; transpose q; compact the key dim to unmasked keys."""
    q = np.asarray(q, dtype=np.float32)
    k = np.asarray(k, dtype=np.float32)
    v = np.asarray(v, dtype=np.float32)
    mask = np.asarray(mask, dtype=np.int32).reshape(len(q), -1)

    B = len(q)
    idxs = [np.nonzero(mask[b])[0] for b in range(B)]
    n_eff = max((len(ix) for ix in idxs), default=1)
    sk = max(P, ((n_eff + P - 1) // P) * P)  # padded compacted key length

    kg = np.zeros((B, D, sk), dtype=np.float32)
    vg = np.zeros((B, sk, D), dtype=np.float32)
    # exp bias: -SHIFT for real keys, -1e9 for padding (kills it exactly),
    # laid out [P, sk//P] partition-major to match the k-tile slicing
    bg = np.full((B, sk), -1.0e9, dtype=np.float32)
    for b in range(B):
        ix = idxs[b]
        kg[b, :, : len(ix)] = k[b][:, ix]
        vg[b, : len(ix)] = v[b][ix]
        bg[b, : len(ix)] = -SHIFT
    bgt = np.ascontiguousarray(
        bg.reshape(B, sk // P, P).transpose(0, 2, 1)
    )  # [B, P, nkt]

    in_maps = []
    for i in range(NCORES):
        sl = slice(i * NB, (i + 1) * NB)
        in_maps.append(
            {
                "qT": np.ascontiguousarray(
                    np.transpose(q[sl], (0, 2, 1)).astype(BF16NP)
                ),
                "k": np.ascontiguousarray(kg[sl].astype(BF16NP)),
                "v": np.ascontiguousarray(vg[sl].astype(BF16NP)),
                "bias": np.ascontiguousarray(bgt[sl]),
            }
        )
    return in_maps, sk


def run(q, k, v, mask, **kwargs):
    in_maps, sk = make_in_maps(q, k, v, mask)
    nc = build(sk)
    nc.finalize()  # run the Bacc pass pipeline (reg alloc, wait splitting)
    res = run_bass_kernel_spmd(nc, in_maps, list(range(NCORES)), **kwargs)
    out = np.concatenate([r["out"] for r in res.results], axis=0)
    return out, res


def kernel(q, k, v, mask):
    out, _ = run(q, k, v, mask)
    return out
